# revision 1
# baseline (speedup 1.0000x reference)
"""Causal self-attention (B=4, T=2048, C=1024, 16 heads) on 8 trn2 NeuronCores.

Sharding: core c handles batch b = c//2 and head-group hg = c%2 (8 of 16 heads).
Each core computes QKV projection for its heads, causal attention, and a partial
output projection (row-sharded W_proj); the host sums the two partials per batch
and adds b_proj.

Device layout notes:
 - x is fed pre-transposed ([C, T]) so the contraction dim C lands on SBUF
   partitions with no on-device transpose.
 - Scores are computed transposed (S^T[k, q]) so softmax's reduction over k can
   be done by the PE via a ones-column appended to V (row k of S^T is a
   partition; summing over partitions is a matmul).
 - Softmax skips the max-subtraction: scores/8 are ~N(0,1) here, exp is safe in
   fp32 and the result is mathematically identical.
 - q/k/scores matmuls run in float32r; V, exp output, yn and W_proj are bf16
   (same PE speed, half the SBUF/LDWEIGHTS traffic; end-to-end rel err ~4e-3).

Performance structure (v4) — built around the TRN2 PE DVFS behaviour (clock
ramps 0.65 -> 1.2 -> 2.4 GHz only after ~3us of *gapless* execution; any idle
gap resets it):
 - Phase 1a (QKV for token ranges 0,1 = tokens 0..1023): chunk-outer loop over
   8 PSUM banks so the first matmul starts after ~1MB of DMA, then gapless PE.
 - Phase 2 attention: per head, q in two 1024-col halves, kt inner. Scores
   psum [128,1024] double-buffered; y accumulates in py psum [128,1024]
   (rows 0..63 y, row 64 rowsum via the V ones-column).
 - QKV for ranges 2,3 (tokens 1024..2047) is *interleaved into half-0's
   attention stream* (their accumulators rotate through the py psum pool):
   the attention units alone leave the PE ~250ns idle per unit (the ACT
   engine's per-exp overhead), which would pin the DVFS at 1.2GHz; the
   interleaved QKV keeps the PE queue full so it runs at 2.4GHz.
 - Normalization (no PE involvement): rowsum staged to SBUF (the custom-DVE
   reciprocal's bit-trick seed misreads PSUM), reciprocal_approx_fast (DVE),
   partition_broadcast (GpSimd), then one DVE multiply -> yn (bf16).
 - Output projection of half 0 drips into half 1's attention stream using the
   scores psum pool slots; half 1's projection is the tail. Output DMA
   overlaps compute.
"""
import numpy as np

T = 2048          # tokens per batch element
C = 1024          # embed dim
H = 8             # heads per core
D = 64            # head dim
CC = 8            # contraction chunks (C / 128)

_CACHE = {}


def _build_nc():
    from concourse import bacc
    import concourse.mybir as mybir
    import concourse.tile as tile

    f32 = mybir.dt.float32
    f32r = mybir.dt.float32r
    bf16 = mybir.dt.bfloat16
    EXP = mybir.ActivationFunctionType.Exp

    nc = bacc.Bacc("TRN2", num_devices=8, debug=False)

    xt_d = nc.dram_tensor("xt", [C, T], f32r, kind="ExternalInput")
    wqkv_d = nc.dram_tensor("wqkv", [C, 1536], f32r, kind="ExternalInput")
    bqk_d = nc.dram_tensor("bqk", [128, 8], f32, kind="ExternalInput")
    bv_d = nc.dram_tensor("bv", [1, 512], f32r, kind="ExternalInput")
    wproj_d = nc.dram_tensor("wproj", [512, C], bf16, kind="ExternalInput")
    ones_d = nc.dram_tensor("ones", [1, 128], f32r, kind="ExternalInput")
    maskb_d = nc.dram_tensor("maskb", [128, 256], bf16, kind="ExternalInput")
    out_d = nc.dram_tensor("out", [T, C], f32, kind="ExternalOutput")

    with tile.TileContext(nc) as tc:
      with tc.tile_pool(name="persist", bufs=1) as pp:
        # persistent SBUF: qk^T [1024 feats, T] f32r, v [T, 8*(64+1)] bf16
        qk_sb = [pp.tile([128, T], f32r, tag=f"qk{f}", name=f"qk{f}") for f in range(8)]
        v_sb = [pp.tile([128, H * 65], bf16, tag=f"v{t}", name=f"v{t}") for t in range(16)]
        wp_sb = [pp.tile([128, C], bf16, tag=f"wp{i}", name=f"wp{i}") for i in range(4)]
        maskb_sb = pp.tile([128, 256], bf16, tag="maskb")
        ones_sb = pp.tile([1, 128], f32r, tag="ones")
        bqk_sb = pp.tile([128, 8], f32, tag="bqk")
        bv_sb = pp.tile([1, 512], f32r, tag="bv")

        nc.sync.dma_start(maskb_sb[:], maskb_d[:])
        nc.sync.dma_start(ones_sb[:], ones_d[:])
        nc.sync.dma_start(bqk_sb[:], bqk_d[:])
        nc.sync.dma_start(bv_sb[:], bv_d[:])
        for i in range(4):
            nc.sync.dma_start(wp_sb[i][:], wproj_d[i * 128:(i + 1) * 128, :])
        for t in range(16):
            # ones column at position 64 of each head's 65-wide V block
            nc.gpsimd.memset(
                v_sb[t][:].rearrange("p (h e) -> p h e", e=65)[:, :, 64:65], 1.0
            )

        def emit_qk_feature(pool, f, xs2, dst):
            # q/k features f*128..f*128+128 for tokens dst..dst+1024
            # (xs2 = two lists of 512-col x chunk tiles)
            pq = pool.tile([128, 1024], f32, tag="py", name="pq")
            for h, xs in enumerate(xs2):
                for c in range(CC):
                    nc.tensor.matmul(
                        pq[:, h * 512:(h + 1) * 512],
                        w_sb[c][:, f * 128:(f + 1) * 128],
                        xs[c][:],
                        start=(c == 0), stop=(c == CC - 1),
                    )
            nc.vector.tensor_scalar_add(
                qk_sb[f][:, dst:dst + 1024], pq[:], bqk_sb[:, f:f + 1]
            )

        def emit_v_tile(pool, tl, xs2, tg):
            # v for 128 tokens (tl-th 128-block of xs2) -> v_sb[tg]
            xs = xs2[tl // 4]
            t0 = (tl % 4) * 128
            pv = pool.tile([128, 1024], f32, tag="py", name="pv")
            for c in range(CC):
                nc.tensor.matmul(
                    pv[:, 0:512], xs[c][:, t0:t0 + 128],
                    w_sb[c][:, 1024:1536],
                    start=(c == 0), stop=False,
                )
            nc.tensor.matmul(pv[:, 0:512], ones_sb[:], bv_sb[:], start=False, stop=True)
            nc.vector.tensor_copy(
                v_sb[tg][:].rearrange("p (h e) -> p h e", e=65)[:, :, 0:64],
                pv[:, 0:512].rearrange("p (h e) -> p h e", e=64),
            )

        # ---------------- Phase 1a: QKV for tokens 0..1023 (ranges 0,1) -----
        with (
            tc.tile_pool(name="ynp", bufs=2) as ynp,
            tc.tile_pool(name="epool", bufs=3) as ep,
            tc.tile_pool(name="rpool", bufs=1) as rp,
        ):
          yn_cur = {}
          py_cur = {}
          proj_q = []
          with (
            tc.tile_pool(name="w", bufs=1) as pw,
            tc.tile_pool(name="xa", bufs=2) as pxa,
          ):
            w_sb = [pw.tile([128, 1536], f32r, tag=f"w{c}", name=f"w{c}") for c in range(CC)]
            with (
                tc.tile_pool(name="psA", bufs=1, space="PSUM") as psA,
            ):
                x_r = {}
                for c in range(CC):
                    # pair chunk DMAs so the first matmul group starts early
                    nc.sync.dma_start(w_sb[c][:], wqkv_d[c * 128:(c + 1) * 128, :])
                    t_ = pxa.tile([128, 512], f32r, tag=f"x{c}", name=f"x{c}")
                    nc.sync.dma_start(t_[:], xt_d[c * 128:(c + 1) * 128, 0:512])
                    x_r.setdefault(0, []).append(t_)
                for r in (0, 1):
                    if r == 1:
                        x_r[1] = []
                        for c in range(CC):
                            t_ = pxa.tile([128, 512], f32r, tag=f"x{c}", name=f"x{c}")
                            nc.sync.dma_start(
                                t_[:], xt_d[c * 128:(c + 1) * 128, 512:1024])
                            x_r[1].append(t_)
                    # chunk-outer over 8 psum banks: chunk c usable on arrival
                    pq8 = [psA.tile([128, 512], f32, tag=f"b{f}", name=f"b{f}")
                           for f in range(8)]
                    for c in range(CC):
                        for f in range(8):
                            nc.tensor.matmul(
                                pq8[f][:], w_sb[c][:, f * 128:(f + 1) * 128],
                                x_r[r][c][:],
                                start=(c == 0), stop=(c == CC - 1),
                            )
                    for f in range(8):
                        nc.vector.tensor_scalar_add(
                            qk_sb[f][:, r * 512:(r + 1) * 512], pq8[f][:],
                            bqk_sb[:, f:f + 1],
                        )
                    for tl in range(4):
                        tg = r * 4 + tl
                        pv = psA.tile([128, 512], f32, tag=f"b{tl}", name=f"pv{tl}")
                        for c in range(CC):
                            nc.tensor.matmul(
                                pv[:], x_r[r][c][:, tl * 128:(tl + 1) * 128],
                                w_sb[c][:, 1024:1536],
                                start=(c == 0), stop=False,
                            )
                        nc.tensor.matmul(pv[:], ones_sb[:], bv_sb[:],
                                         start=False, stop=True)
                        nc.vector.tensor_copy(
                            v_sb[tg][:].rearrange("p (h e) -> p h e", e=65)[:, :, 0:64],
                            pv[:].rearrange("p (h e) -> p h e", e=64),
                        )

            # x for tokens 1024..2047 (ranges 2,3), used by the interleaved
            # QKV: two more generations of the xa pool's chunk tiles
            x23 = []
            for h, lo in enumerate((1024, 1536)):
                xs = []
                for c in range(CC):
                    t_ = pxa.tile([128, 512], f32r, tag=f"x{c}", name=f"x{c}")
                    nc.sync.dma_start(t_[:], xt_d[c * 128:(c + 1) * 128, lo:lo + 512])
                    xs.append(t_)
                x23.append(xs)

            # ---------------- Phase 2: attention ----------------------------
            # half 0 (+ interleaved QKV for ranges 2,3), then half 1 (+ proj)
            if True:
                def emit_scores(pss, u):
                    half, head, kt = u
                    fq = head // 2
                    row = (head % 2) * 64
                    qT = qk_sb[fq]
                    kT = qk_sb[4 + fq]
                    a_rel = max(0, kt * 128 - 1024 * half)
                    diag_kt = kt >= 8 * half
                    ps = pss.tile([128, 1024], f32, tag="s", name="ps_s")
                    for b in (0, 1):
                        lo, hi = 512 * b, 512 * b + 512
                        if hi <= a_rel:
                            continue
                        gs = max(lo, a_rel)
                        has_diag = diag_kt and lo <= a_rel < hi
                        nc.tensor.matmul(
                            ps[:, gs:hi],
                            kT[row:row + 64, kt * 128:(kt + 1) * 128],
                            qT[row:row + 64, 1024 * half + gs:1024 * half + hi],
                            start=True, stop=not has_diag,
                        )
                        if has_diag:
                            # += -1e30 * upper_strict on the diag block
                            nc.tensor.matmul(
                                ps[:, a_rel:a_rel + 128],
                                maskb_sb[:, 0:128], maskb_sb[:, 128:256],
                                start=False, stop=True,
                            )
                    return ps

                def emit_exp(u, ps):
                    half, head, kt = u
                    a_rel = max(0, kt * 128 - 1024 * half)
                    e = ep.tile([128, 1024], bf16, tag="e", name="e_t")
                    nc.scalar.activation(
                        e[:, 0:1024 - a_rel], ps[:, a_rel:1024], EXP, scale=0.125,
                    )
                    return e

                def emit_y(psy, u, e):
                    half, head, kt = u
                    a_rel = max(0, kt * 128 - 1024 * half)
                    if kt == 0:
                        py_cur[(half, head)] = psy.tile(
                            [128, 1024], f32, tag="py", name="py_t")
                    py = py_cur[(half, head)]
                    for b in (0, 1):
                        lo, hi = 512 * b, 512 * b + 512
                        if hi <= a_rel:
                            continue
                        gs = max(lo, a_rel)
                        nc.tensor.matmul(
                            py[0:65, gs:hi],
                            v_sb[kt][:, head * 65:(head + 1) * 65],
                            e[:, gs - a_rel:hi - a_rel],
                            start=(kt == 0), stop=(kt == 8 * half + 4 * b + 3),
                        )

                def emit_norm(u):
                    half, head, _ = u
                    fq = head // 2
                    row = (head % 2) * 64
                    py = py_cur[(half, head)]
                    # stage rowsum to SBUF: the custom-DVE recip's bit-trick
                    # seed misreads PSUM
                    rs = rp.tile([1, 1024], f32, tag="rs", name="rs_t")
                    nc.vector.tensor_copy(rs[:], py[64:65, 0:1024])
                    r = rp.tile([1, 1024], f32, tag="r", name="r_t")
                    nc.vector.reciprocal_approx_fast(r[:], rs[:])
                    rb = rp.tile([64, 1024], f32, tag="rb", name="rb_t")
                    nc.gpsimd.partition_broadcast(rb[:], r[:])
                    nc.vector.tensor_mul(
                        yn_cur[half][fq][row:row + 64, :], py[0:64, :], rb[:],
                    )
                    if head == H - 1:
                        for tt in range(8):
                            proj_q.append((half, tt))

                def emit_proj_tt(pss, obp):
                    half, tt = proj_q.pop(0)
                    po = pss.tile([128, 1024], f32, tag="s", name="po_t")
                    for fc in range(4):
                        for n in range(2):
                            nc.tensor.matmul(
                                po[:, n * 512:(n + 1) * 512],
                                yn_cur[half][fc][:, tt * 128:(tt + 1) * 128],
                                wp_sb[fc][:, n * 512:(n + 1) * 512],
                                start=(fc == 0), stop=(fc == 3),
                            )
                    ob = obp.tile([128, C], f32, tag="ob")
                    nc.vector.tensor_copy(ob[:], po[:])
                    nc.sync.dma_start(
                        out_d[half * 1024 + tt * 128:half * 1024 + (tt + 1) * 128, :],
                        ob[:],
                    )

                def run_stream(pss, psy, units, fillers, obp=None, proj_every=16):
                    # scores(i+1) traced before y(i). Fillers (independent PE
                    # work) are emitted ONLY at block boundaries: the psy pool
                    # slot they rotate through frees exactly there (a filler
                    # allocated mid-block would wait on a py slot that only
                    # frees at the block's norm, head-of-line-blocking the PE).
                    fq_ = list(fillers)
                    ps_i = emit_scores(pss, units[0])
                    for i, u in enumerate(units):
                        half, head, kt = u
                        if half not in yn_cur:
                            yn_cur[half] = [
                                ynp.tile([128, 1024], bf16, tag=f"yn{fc}", name=f"yn{fc}")
                                for fc in range(4)
                            ]
                        e_i = emit_exp(u, ps_i)
                        if i + 1 < len(units):
                            ps_i = emit_scores(pss, units[i + 1])
                        emit_y(psy, u, e_i)
                        if kt == 8 * half + 7:
                            emit_norm(u)
                            for _ in range(2):
                                if fq_:
                                    fq_.pop(0)(psy)
                        if obp is not None and proj_q and i % proj_every == 0:
                            emit_proj_tt(pss, obp)
                    while fq_:
                        fq_.pop(0)(psy)

                units0 = [(0, head, kt) for head in range(H) for kt in range(8)]
                units1 = [(1, head, kt) for head in range(H) for kt in range(16)]

                # interleaved QKV work items for tokens 1024..2047
                fillers = []
                for f in range(8):
                    fillers.append(lambda p, f=f: emit_qk_feature(p, f, x23, 1024))
                for tl in range(8):
                    fillers.append(lambda p, tl=tl: emit_v_tile(p, tl, x23, 8 + tl))

                with (
                    tc.tile_pool(name="pss0", bufs=2, space="PSUM") as pss0,
                    tc.tile_pool(name="psy0", bufs=2, space="PSUM") as psy0,
                ):
                    run_stream(pss0, psy0, units0, fillers)

          # w and x23 pools closed; half 1 needs neither
          with (
              tc.tile_pool(name="obp", bufs=2) as obp,
              tc.tile_pool(name="pss1", bufs=2, space="PSUM") as pss1,
              tc.tile_pool(name="psy1", bufs=2, space="PSUM") as psy1,
          ):
              run_stream(pss1, psy1, units1, [], obp=obp, proj_every=16)
              while proj_q:
                  emit_proj_tt(pss1, obp)

    nc.compile()
    return nc


def _get_nc():
    if "nc" not in _CACHE:
        _CACHE["nc"] = _build_nc()
    return _CACHE["nc"]


def prepare_in_maps(x, W_attn, b_attn, W_proj, b_proj):
    import ml_dtypes
    x = np.asarray(x, dtype=np.float32)
    W_attn = np.asarray(W_attn, dtype=np.float32)
    b_attn = np.asarray(b_attn, dtype=np.float32)
    W_proj = np.asarray(W_proj, dtype=np.float32)

    mask = np.zeros((128, 256), np.float32)
    mask[:, 0:128] = np.triu(np.ones((128, 128), np.float32), 1)
    mask[:, 128:256] = -1e30 * np.eye(128, dtype=np.float32)
    maskb = np.ascontiguousarray(mask.astype(ml_dtypes.bfloat16))
    ones = np.ones((1, 128), np.float32)
    xts = [np.ascontiguousarray(x[b].T) for b in range(4)]

    in_maps = []
    for c in range(8):
        b, hg = divmod(c, 2)
        s = hg * 512
        wqkv = np.ascontiguousarray(np.concatenate(
            [W_attn[:, s:s + 512],
             W_attn[:, 1024 + s:1024 + s + 512],
             W_attn[:, 2048 + s:2048 + s + 512]], axis=1))
        bqk = np.ascontiguousarray(
            np.concatenate([b_attn[s:s + 512], b_attn[1024 + s:1024 + s + 512]])
            .reshape(8, 128).T)
        bv = np.ascontiguousarray(b_attn[2048 + s:2048 + s + 512].reshape(1, 512))
        wproj = np.ascontiguousarray(
            W_proj[s:s + 512, :].astype(ml_dtypes.bfloat16))
        in_maps.append({"xt": xts[b], "wqkv": wqkv, "bqk": bqk, "bv": bv,
                        "wproj": wproj, "ones": ones, "maskb": maskb})
    return in_maps


def kernel(x, W_attn, b_attn, W_proj, b_proj):
    from concourse.bass_utils import run_bass_kernel_spmd

    b_proj = np.asarray(b_proj, dtype=np.float32)
    nc = _get_nc()
    in_maps = prepare_in_maps(x, W_attn, b_attn, W_proj, b_proj)

    res = run_bass_kernel_spmd(nc, in_maps, core_ids=list(range(8)))
    y = np.empty((4, T, C), np.float32)
    for b in range(4):
        y[b] = res.results[2 * b]["out"] + res.results[2 * b + 1]["out"] + b_proj
    return y



# revision 2
# speedup vs baseline: 1.0982x; 1.0982x over previous
"""Causal self-attention (B=4, T=2048, C=1024, 16 heads) on 8 trn2 NeuronCores.

Sharding: core c handles batch b = c//2 and head-group hg = c%2 (8 of 16 heads).
Each core computes QKV projection for its heads, causal attention, and a partial
output projection (row-sharded W_proj); the host sums the two partials per batch
and adds b_proj.

Device layout notes:
 - x is fed pre-transposed ([C, T]) so the contraction dim C lands on SBUF
   partitions with no on-device transpose.
 - Scores are computed transposed (S^T[k, q]) so softmax's reduction over k can
   be done by the PE via a ones-column appended to V (row k of S^T is a
   partition; summing over partitions is a matmul).
 - Softmax skips the max-subtraction: scores/8 are ~N(0,1) here, exp is safe in
   fp32 and the result is mathematically identical.
 - q/k/scores matmuls run in float32r; V, exp output, yn and W_proj are bf16
   (same PE speed, half the SBUF/LDWEIGHTS traffic; end-to-end rel err ~4e-3).

Performance structure (v4) — built around the TRN2 PE DVFS behaviour (clock
ramps 0.65 -> 1.2 -> 2.4 GHz only after ~3us of *gapless* execution; any idle
gap resets it):
 - Phase 1a (QKV for token ranges 0,1 = tokens 0..1023): chunk-outer loop over
   8 PSUM banks so the first matmul starts after ~1MB of DMA, then gapless PE.
 - Phase 2 attention: per head, q in two 1024-col halves, kt inner. Scores
   psum [128,1024] double-buffered; y accumulates in py psum [128,1024]
   (rows 0..63 y, row 64 rowsum via the V ones-column).
 - QKV for ranges 2,3 (tokens 1024..2047) is *interleaved into half-0's
   attention stream* (their accumulators rotate through the py psum pool):
   the attention units alone leave the PE ~250ns idle per unit (the ACT
   engine's per-exp overhead), which would pin the DVFS at 1.2GHz; the
   interleaved QKV keeps the PE queue full so it runs at 2.4GHz.
 - Normalization (no PE involvement): rowsum staged to SBUF (the custom-DVE
   reciprocal's bit-trick seed misreads PSUM), reciprocal_approx_fast (DVE),
   partition_broadcast (GpSimd), then one DVE multiply -> yn (bf16).
 - Output projection of half 0 drips into half 1's attention stream using the
   scores psum pool slots; half 1's projection is the tail. Output DMA
   overlaps compute.
"""
import numpy as np

T = 2048          # tokens per batch element
C = 1024          # embed dim
H = 8             # heads per core
D = 64            # head dim
CC = 8            # contraction chunks (C / 128)

_CACHE = {}


def _build_nc():
    from concourse import bacc
    import concourse.mybir as mybir
    import concourse.tile as tile

    f32 = mybir.dt.float32
    f32r = mybir.dt.float32r
    f16 = mybir.dt.float16
    bf16 = mybir.dt.bfloat16
    EXP = mybir.ActivationFunctionType.Exp

    nc = bacc.Bacc("TRN2", num_devices=8, debug=False)

    xt_d = nc.dram_tensor("xt", [C, T], f16, kind="ExternalInput")
    wqkv_d = nc.dram_tensor("wqkv", [C, 1536], f16, kind="ExternalInput")
    bqk_d = nc.dram_tensor("bqk", [128, 8], f32, kind="ExternalInput")
    bv_d = nc.dram_tensor("bv", [1, 512], f16, kind="ExternalInput")
    wproj_d = nc.dram_tensor("wproj", [512, C], bf16, kind="ExternalInput")
    ones_d = nc.dram_tensor("ones", [1, 128], f16, kind="ExternalInput")
    maskb_d = nc.dram_tensor("maskb", [128, 256], bf16, kind="ExternalInput")
    out_d = nc.dram_tensor("out", [T, C], f32, kind="ExternalOutput")

    with tile.TileContext(nc) as tc:
      with tc.tile_pool(name="persist", bufs=1) as pp:
        # persistent SBUF: qk^T [1024 feats, T] f32r, v [T, 8*(64+1)] bf16
        qk_sb = [pp.tile([128, T], f16, tag=f"qk{f}", name=f"qk{f}") for f in range(8)]
        v_sb = [pp.tile([128, H * 65], bf16, tag=f"v{t}", name=f"v{t}") for t in range(16)]
        wp_sb = [pp.tile([128, C], bf16, tag=f"wp{i}", name=f"wp{i}") for i in range(4)]
        maskb_sb = pp.tile([128, 256], bf16, tag="maskb")
        ones_sb = pp.tile([1, 128], f16, tag="ones")
        bqk_sb = pp.tile([128, 8], f32, tag="bqk")
        bv_sb = pp.tile([1, 512], f16, tag="bv")

        nc.sync.dma_start(maskb_sb[:], maskb_d[:])
        nc.sync.dma_start(ones_sb[:], ones_d[:])
        nc.sync.dma_start(bqk_sb[:], bqk_d[:])
        nc.sync.dma_start(bv_sb[:], bv_d[:])
        for i in range(4):
            nc.sync.dma_start(wp_sb[i][:], wproj_d[i * 128:(i + 1) * 128, :])
        for t in range(16):
            # ones column at position 64 of each head's 65-wide V block
            nc.gpsimd.memset(
                v_sb[t][:].rearrange("p (h e) -> p h e", e=65)[:, :, 64:65], 1.0
            )

        def emit_qk_feature(pool, f, xs2, dst):
            # q/k features f*128..f*128+128 for tokens dst..dst+1024
            # (xs2 = two lists of 512-col x chunk tiles)
            pq = pool.tile([128, 1024], f32, tag="py", name="pq")
            for h, xs in enumerate(xs2):
                for c in range(CC):
                    nc.tensor.matmul(
                        pq[:, h * 512:(h + 1) * 512],
                        w_sb[c][:, f * 128:(f + 1) * 128],
                        xs[c][:],
                        start=(c == 0), stop=(c == CC - 1),
                    )
            nc.vector.tensor_scalar_add(
                qk_sb[f][:, dst:dst + 1024], pq[:], bqk_sb[:, f:f + 1]
            )

        def emit_v_tile(pool, tl, xs2, tg):
            # v for 128 tokens (tl-th 128-block of xs2) -> v_sb[tg]
            xs = xs2[tl // 4]
            t0 = (tl % 4) * 128
            pv = pool.tile([128, 1024], f32, tag="py", name="pv")
            for c in range(CC):
                nc.tensor.matmul(
                    pv[:, 0:512], xs[c][:, t0:t0 + 128],
                    w_sb[c][:, 1024:1536],
                    start=(c == 0), stop=False,
                )
            nc.tensor.matmul(pv[:, 0:512], ones_sb[:], bv_sb[:], start=False, stop=True)
            nc.vector.tensor_copy(
                v_sb[tg][:].rearrange("p (h e) -> p h e", e=65)[:, :, 0:64],
                pv[:, 0:512].rearrange("p (h e) -> p h e", e=64),
            )

        # ---------------- Phase 1a: QKV for tokens 0..1023 (ranges 0,1) -----
        with (
            tc.tile_pool(name="ynp", bufs=2) as ynp,
            tc.tile_pool(name="epool", bufs=3) as ep,
            tc.tile_pool(name="rpool", bufs=1) as rp,
        ):
          yn_cur = {}
          py_cur = {}
          proj_q = []
          with (
            tc.tile_pool(name="w", bufs=1) as pw,
            tc.tile_pool(name="xa", bufs=2) as pxa,
          ):
            w_sb = [pw.tile([128, 1536], f16, tag=f"w{c}", name=f"w{c}") for c in range(CC)]
            with (
                tc.tile_pool(name="psA", bufs=1, space="PSUM") as psA,
            ):
                x_r = {}
                for c in range(CC):
                    # pair chunk DMAs so the first matmul group starts early
                    nc.sync.dma_start(w_sb[c][:], wqkv_d[c * 128:(c + 1) * 128, :])
                    t_ = pxa.tile([128, 512], f16, tag=f"x{c}", name=f"x{c}")
                    nc.sync.dma_start(t_[:], xt_d[c * 128:(c + 1) * 128, 0:512])
                    x_r.setdefault(0, []).append(t_)
                for r in (0, 1):
                    if r == 1:
                        x_r[1] = []
                        for c in range(CC):
                            t_ = pxa.tile([128, 512], f16, tag=f"x{c}", name=f"x{c}")
                            nc.sync.dma_start(
                                t_[:], xt_d[c * 128:(c + 1) * 128, 512:1024])
                            x_r[1].append(t_)
                    # chunk-outer over 8 psum banks: chunk c usable on arrival
                    pq8 = [psA.tile([128, 512], f32, tag=f"b{f}", name=f"b{f}")
                           for f in range(8)]
                    for c in range(CC):
                        for f in range(8):
                            nc.tensor.matmul(
                                pq8[f][:], w_sb[c][:, f * 128:(f + 1) * 128],
                                x_r[r][c][:],
                                start=(c == 0), stop=(c == CC - 1),
                            )
                    for f in range(8):
                        nc.vector.tensor_scalar_add(
                            qk_sb[f][:, r * 512:(r + 1) * 512], pq8[f][:],
                            bqk_sb[:, f:f + 1],
                        )
                    for tl in range(4):
                        tg = r * 4 + tl
                        pv = psA.tile([128, 512], f32, tag=f"b{tl}", name=f"pv{tl}")
                        for c in range(CC):
                            nc.tensor.matmul(
                                pv[:], x_r[r][c][:, tl * 128:(tl + 1) * 128],
                                w_sb[c][:, 1024:1536],
                                start=(c == 0), stop=False,
                            )
                        nc.tensor.matmul(pv[:], ones_sb[:], bv_sb[:],
                                         start=False, stop=True)
                        nc.vector.tensor_copy(
                            v_sb[tg][:].rearrange("p (h e) -> p h e", e=65)[:, :, 0:64],
                            pv[:].rearrange("p (h e) -> p h e", e=64),
                        )

            # x for tokens 1024..2047 (ranges 2,3), used by the interleaved
            # QKV: two more generations of the xa pool's chunk tiles
            x23 = []
            for h, lo in enumerate((1024, 1536)):
                xs = []
                for c in range(CC):
                    t_ = pxa.tile([128, 512], f16, tag=f"x{c}", name=f"x{c}")
                    nc.sync.dma_start(t_[:], xt_d[c * 128:(c + 1) * 128, lo:lo + 512])
                    xs.append(t_)
                x23.append(xs)

            # ---------------- Phase 2: attention ----------------------------
            # half 0 (+ interleaved QKV for ranges 2,3), then half 1 (+ proj)
            if True:
                def emit_scores(pss, u):
                    half, head, kt = u
                    fq = head // 2
                    row = (head % 2) * 64
                    qT = qk_sb[fq]
                    kT = qk_sb[4 + fq]
                    a_rel = max(0, kt * 128 - 1024 * half)
                    diag_kt = kt >= 8 * half
                    ps = pss.tile([128, 1024], f32, tag="s", name="ps_s")
                    for b in (0, 1):
                        lo, hi = 512 * b, 512 * b + 512
                        if hi <= a_rel:
                            continue
                        gs = max(lo, a_rel)
                        has_diag = diag_kt and lo <= a_rel < hi
                        nc.tensor.matmul(
                            ps[:, gs:hi],
                            kT[row:row + 64, kt * 128:(kt + 1) * 128],
                            qT[row:row + 64, 1024 * half + gs:1024 * half + hi],
                            start=True, stop=not has_diag,
                        )
                        if has_diag:
                            # += -1e30 * upper_strict on the diag block
                            nc.tensor.matmul(
                                ps[:, a_rel:a_rel + 128],
                                maskb_sb[:, 0:128], maskb_sb[:, 128:256],
                                start=False, stop=True,
                            )
                    return ps

                def emit_exp(u, ps):
                    half, head, kt = u
                    a_rel = max(0, kt * 128 - 1024 * half)
                    e = ep.tile([128, 1024], bf16, tag="e", name="e_t")
                    nc.scalar.activation(
                        e[:, 0:1024 - a_rel], ps[:, a_rel:1024], EXP, scale=0.125,
                    )
                    return e

                def emit_y(psy, u, e):
                    half, head, kt = u
                    a_rel = max(0, kt * 128 - 1024 * half)
                    if kt == 0:
                        py_cur[(half, head)] = psy.tile(
                            [128, 1024], f32, tag="py", name="py_t")
                    py = py_cur[(half, head)]
                    for b in (0, 1):
                        lo, hi = 512 * b, 512 * b + 512
                        if hi <= a_rel:
                            continue
                        gs = max(lo, a_rel)
                        nc.tensor.matmul(
                            py[0:65, gs:hi],
                            v_sb[kt][:, head * 65:(head + 1) * 65],
                            e[:, gs - a_rel:hi - a_rel],
                            start=(kt == 0), stop=(kt == 8 * half + 4 * b + 3),
                        )

                def emit_norm(u):
                    half, head, _ = u
                    fq = head // 2
                    row = (head % 2) * 64
                    py = py_cur[(half, head)]
                    # stage rowsum to SBUF: the custom-DVE recip's bit-trick
                    # seed misreads PSUM
                    rs = rp.tile([1, 1024], f32, tag="rs", name="rs_t")
                    nc.vector.tensor_copy(rs[:], py[64:65, 0:1024])
                    r = rp.tile([1, 1024], f32, tag="r", name="r_t")
                    nc.vector.reciprocal_approx_fast(r[:], rs[:])
                    rb = rp.tile([64, 1024], f32, tag="rb", name="rb_t")
                    nc.gpsimd.partition_broadcast(rb[:], r[:])
                    nc.vector.tensor_mul(
                        yn_cur[half][fq][row:row + 64, :], py[0:64, :], rb[:],
                    )
                    if head == H - 1:
                        for tt in range(8):
                            proj_q.append((half, tt))

                def emit_proj_tt(pss, obp):
                    half, tt = proj_q.pop(0)
                    po = pss.tile([128, 1024], f32, tag="s", name="po_t")
                    for fc in range(4):
                        for n in range(2):
                            nc.tensor.matmul(
                                po[:, n * 512:(n + 1) * 512],
                                yn_cur[half][fc][:, tt * 128:(tt + 1) * 128],
                                wp_sb[fc][:, n * 512:(n + 1) * 512],
                                start=(fc == 0), stop=(fc == 3),
                            )
                    ob = obp.tile([128, C], f32, tag="ob")
                    nc.vector.tensor_copy(ob[:], po[:])
                    nc.sync.dma_start(
                        out_d[half * 1024 + tt * 128:half * 1024 + (tt + 1) * 128, :],
                        ob[:],
                    )

                def run_stream(pss, psy, units, fillers, obp=None, proj_every=16):
                    # scores(i+1) traced before y(i). Fillers (independent PE
                    # work) are emitted ONLY at block boundaries: the psy pool
                    # slot they rotate through frees exactly there (a filler
                    # allocated mid-block would wait on a py slot that only
                    # frees at the block's norm, head-of-line-blocking the PE).
                    fq_ = list(fillers)
                    ps_i = emit_scores(pss, units[0])
                    for i, u in enumerate(units):
                        half, head, kt = u
                        if half not in yn_cur:
                            yn_cur[half] = [
                                ynp.tile([128, 1024], bf16, tag=f"yn{fc}", name=f"yn{fc}")
                                for fc in range(4)
                            ]
                        e_i = emit_exp(u, ps_i)
                        if i + 1 < len(units):
                            ps_i = emit_scores(pss, units[i + 1])
                        emit_y(psy, u, e_i)
                        if kt == 8 * half + 7:
                            emit_norm(u)
                            for _ in range(2):
                                if fq_:
                                    fq_.pop(0)(psy)
                        if obp is not None and proj_q and i % proj_every == 0:
                            emit_proj_tt(pss, obp)
                    while fq_:
                        fq_.pop(0)(psy)

                units0 = [(0, head, kt) for head in range(H) for kt in range(8)]
                units1 = [(1, head, kt) for head in range(H) for kt in range(16)]

                # interleaved QKV work items for tokens 1024..2047
                fillers = []
                for f in range(8):
                    fillers.append(lambda p, f=f: emit_qk_feature(p, f, x23, 1024))
                for tl in range(8):
                    fillers.append(lambda p, tl=tl: emit_v_tile(p, tl, x23, 8 + tl))

                with (
                    tc.tile_pool(name="pss0", bufs=2, space="PSUM") as pss0,
                    tc.tile_pool(name="psy0", bufs=2, space="PSUM") as psy0,
                ):
                    run_stream(pss0, psy0, units0, fillers)

          # w and x23 pools closed; half 1 needs neither
          with (
              tc.tile_pool(name="obp", bufs=2) as obp,
              tc.tile_pool(name="pss1", bufs=2, space="PSUM") as pss1,
              tc.tile_pool(name="psy1", bufs=2, space="PSUM") as psy1,
          ):
              run_stream(pss1, psy1, units1, [], obp=obp, proj_every=16)
              while proj_q:
                  emit_proj_tt(pss1, obp)

    nc.compile()
    return nc


def _get_nc():
    if "nc" not in _CACHE:
        _CACHE["nc"] = _build_nc()
    return _CACHE["nc"]


def prepare_in_maps(x, W_attn, b_attn, W_proj, b_proj):
    import ml_dtypes
    x = np.asarray(x, dtype=np.float32)
    W_attn = np.asarray(W_attn, dtype=np.float32)
    b_attn = np.asarray(b_attn, dtype=np.float32)
    W_proj = np.asarray(W_proj, dtype=np.float32)

    mask = np.zeros((128, 256), np.float32)
    mask[:, 0:128] = np.triu(np.ones((128, 128), np.float32), 1)
    mask[:, 128:256] = -1e30 * np.eye(128, dtype=np.float32)
    maskb = np.ascontiguousarray(mask.astype(ml_dtypes.bfloat16))
    ones = np.ones((1, 128), np.float16)
    xts = [np.ascontiguousarray(x[b].T.astype(np.float16)) for b in range(4)]

    in_maps = []
    for c in range(8):
        b, hg = divmod(c, 2)
        s = hg * 512
        wqkv = np.ascontiguousarray(np.concatenate(
            [W_attn[:, s:s + 512],
             W_attn[:, 1024 + s:1024 + s + 512],
             W_attn[:, 2048 + s:2048 + s + 512]], axis=1).astype(np.float16))
        bqk = np.ascontiguousarray(
            np.concatenate([b_attn[s:s + 512], b_attn[1024 + s:1024 + s + 512]])
            .reshape(8, 128).T)
        bv = np.ascontiguousarray(b_attn[2048 + s:2048 + s + 512].reshape(1, 512).astype(np.float16))
        wproj = np.ascontiguousarray(
            W_proj[s:s + 512, :].astype(ml_dtypes.bfloat16))
        in_maps.append({"xt": xts[b], "wqkv": wqkv, "bqk": bqk, "bv": bv,
                        "wproj": wproj, "ones": ones, "maskb": maskb})
    return in_maps


def kernel(x, W_attn, b_attn, W_proj, b_proj):
    from concourse.bass_utils import run_bass_kernel_spmd

    b_proj = np.asarray(b_proj, dtype=np.float32)
    nc = _get_nc()
    in_maps = prepare_in_maps(x, W_attn, b_attn, W_proj, b_proj)

    res = run_bass_kernel_spmd(nc, in_maps, core_ids=list(range(8)))
    y = np.empty((4, T, C), np.float32)
    for b in range(4):
        y[b] = res.results[2 * b]["out"] + res.results[2 * b + 1]["out"] + b_proj
    return y



# revision 3
# speedup vs baseline: 1.2088x; 1.1007x over previous
"""Causal self-attention (B=4, T=2048, C=1024, 16 heads) on 8 trn2 NeuronCores.

Sharding: core c handles batch b = c//2 and head-group hg = c%2 (8 of 16 heads).
Each core computes QKV projection for its heads, causal attention, and a partial
output projection (row-sharded W_proj); the host sums the two partials per batch
and adds b_proj.

Device layout notes:
 - x is fed pre-transposed ([C, T]) so the contraction dim C lands on SBUF
   partitions with no on-device transpose.
 - Scores are computed transposed (S^T[k, q]) so softmax's reduction over k can
   be done by the PE via a ones-column appended to V (row k of S^T is a
   partition; summing over partitions is a matmul).
 - Softmax skips the max-subtraction: scores/8 are ~N(0,1) here, exp is safe in
   fp32 and the result is mathematically identical.
 - All matmul operands are fp16 (fp32 PSUM accumulate): same PE stream rate as
   fp32, but FWL (fast weight load) halves LDWEIGHTS time, and SBUF/DMA
   traffic halves. fp16's 11-bit mantissa keeps end-to-end rel err ~3e-3.

Performance structure (v5), built around three engine limits measured in the
v4 trace: PE matmul streaming (~213ns per N=512), per-matmul LDWEIGHTS
serialization, and the ACT engine's exp cost ((N+352)/1.2 ns per instruction):
 - Scores matmuls have K=64 (head dim): the two heads of a feature-pair (fq)
   live on partitions 0-63 / 64-127, so their score MMs target disjoint PE
   row-groups (tile_position auto-derived from base_partition) and run
   CONCURRENTLY when issued back-to-back - halving scores PE time.
 - Attention is organized in pair-units (half, fq, qc, kt) where qc is a
   512-token q chunk: one PSUM tile [128, 1024] holds both heads' scores
   (A: cols 0-511, B: 512-1023), so ONE exp instruction covers two heads
   (fewer ACT fixed overheads). Unwritten diag-trim columns are exp'd as
   garbage but never streamed into the y matmuls.
 - y accumulates per pair-group into a [65, 1024] PSUM region (rows 0-63 y,
   row 64 rowsum via the V ones-column; A cols 0-511, B 512-1023).
 - Normalization per pair-group: rowsum staged to SBUF (the custom-DVE
   reciprocal's bit-trick seed misreads PSUM), reciprocal_approx_fast (DVE),
   partition_broadcast (GpSimd), then two DVE multiplies -> yn (fp16).
 - Phase 1a (QKV for tokens 0..1023): chunk-outer loop over 8 PSUM banks so
   the first matmul starts right after the first w/x chunk DMA lands.
 - QKV for tokens 1024..2047 is interleaved into half-0's attention stream
   (fillers at pair-group boundaries); output projection of half 0 drips into
   half 1's attention stream, half 1's projection is the tail. Output DMA
   overlaps compute. This keeps the PE queue full while ACT paces the
   attention stream, and keeps the PE HAM clock at 2.4GHz.
"""
import numpy as np

T = 2048          # tokens per batch element
C = 1024          # embed dim
H = 8             # heads per core
D = 64            # head dim
CC = 8            # contraction chunks (C / 128)

_CACHE = {}


def _build_nc():
    from concourse import bacc
    import concourse.mybir as mybir
    import concourse.tile as tile

    f32 = mybir.dt.float32
    f16 = mybir.dt.float16
    bf16 = mybir.dt.bfloat16
    EXP = mybir.ActivationFunctionType.Exp

    nc = bacc.Bacc("TRN2", num_devices=8, debug=False)

    xt_d = nc.dram_tensor("xt", [C, T], f16, kind="ExternalInput")
    wqkv_d = nc.dram_tensor("wqkv", [C, 1536], f16, kind="ExternalInput")
    bqk_d = nc.dram_tensor("bqk", [128, 8], f32, kind="ExternalInput")
    bv_d = nc.dram_tensor("bv", [1, 512], f16, kind="ExternalInput")
    wproj_d = nc.dram_tensor("wproj", [512, C], f16, kind="ExternalInput")
    ones_d = nc.dram_tensor("ones", [1, 128], f16, kind="ExternalInput")
    maskb_d = nc.dram_tensor("maskb", [128, 256], bf16, kind="ExternalInput")
    out_d = nc.dram_tensor("out", [T, C], f32, kind="ExternalOutput")

    with tile.TileContext(nc) as tc:
      with tc.tile_pool(name="persist", bufs=1) as pp:
        # persistent SBUF: qk^T [1024 feats, T] f16, v [T, 8*(64+1)] f16
        qk_sb = [pp.tile([128, T], f16, tag=f"qk{f}", name=f"qk{f}") for f in range(8)]
        v_sb = [pp.tile([128, H * 65], f16, tag=f"v{t}", name=f"v{t}") for t in range(16)]
        wp_sb = [pp.tile([128, C], f16, tag=f"wp{i}", name=f"wp{i}") for i in range(4)]
        maskb_sb = pp.tile([128, 256], bf16, tag="maskb")
        ones_sb = pp.tile([1, 128], f16, tag="ones")
        bqk_sb = pp.tile([128, 8], f32, tag="bqk")
        bv_sb = pp.tile([1, 512], f16, tag="bv")

        nc.sync.dma_start(maskb_sb[:], maskb_d[:])
        nc.sync.dma_start(ones_sb[:], ones_d[:])
        nc.sync.dma_start(bqk_sb[:], bqk_d[:])
        nc.sync.dma_start(bv_sb[:], bv_d[:])
        for i in range(4):
            nc.sync.dma_start(wp_sb[i][:], wproj_d[i * 128:(i + 1) * 128, :])
        for t in range(16):
            # ones column at position 64 of each head's 65-wide V block
            nc.gpsimd.memset(
                v_sb[t][:].rearrange("p (h e) -> p h e", e=65)[:, :, 64:65], 1.0
            )

        def emit_qk_feature(pool, f, xs2, dst):
            # q/k features f*128..f*128+128 for tokens dst..dst+1024
            # (xs2 = two lists of 512-col x chunk tiles)
            pq = pool.tile([128, 1024], f32, tag="py", name="pq")
            for h, xs in enumerate(xs2):
                for c in range(CC):
                    nc.tensor.matmul(
                        pq[:, h * 512:(h + 1) * 512],
                        w_sb[c][:, f * 128:(f + 1) * 128],
                        xs[c][:],
                        start=(c == 0), stop=(c == CC - 1),
                    )
            nc.vector.tensor_scalar_add(
                qk_sb[f][:, dst:dst + 1024], pq[:], bqk_sb[:, f:f + 1]
            )

        def emit_v_tile(pool, tl, xs2, tg):
            # v for 128 tokens (tl-th 128-block of xs2) -> v_sb[tg]
            xs = xs2[tl // 4]
            t0 = (tl % 4) * 128
            pv = pool.tile([128, 1024], f32, tag="py", name="pv")
            for c in range(CC):
                nc.tensor.matmul(
                    pv[:, 0:512], xs[c][:, t0:t0 + 128],
                    w_sb[c][:, 1024:1536],
                    start=(c == 0), stop=False,
                )
            nc.tensor.matmul(pv[:, 0:512], ones_sb[:], bv_sb[:], start=False, stop=True)
            nc.vector.tensor_copy(
                v_sb[tg][:].rearrange("p (h e) -> p h e", e=65)[:, :, 0:64],
                pv[:, 0:512].rearrange("p (h e) -> p h e", e=64),
            )

        # ---------------- Phase 1a: QKV for tokens 0..1023 (ranges 0,1) -----
        with (
            tc.tile_pool(name="ynp", bufs=2) as ynp,
            tc.tile_pool(name="epool", bufs=3) as ep,
            tc.tile_pool(name="rpool", bufs=2) as rp,
        ):
          yn_cur = {}
          py_cur = {}
          proj_q = []
          with (
            tc.tile_pool(name="w", bufs=1) as pw,
            tc.tile_pool(name="xa", bufs=2) as pxa,
          ):
            w_sb = [pw.tile([128, 1536], f16, tag=f"w{c}", name=f"w{c}") for c in range(CC)]
            with (
                tc.tile_pool(name="psA", bufs=1, space="PSUM") as psA,
            ):
                x_r = {}
                for c in range(CC):
                    # pair chunk DMAs so the first matmul group starts early
                    nc.sync.dma_start(w_sb[c][:], wqkv_d[c * 128:(c + 1) * 128, :])
                    t_ = pxa.tile([128, 512], f16, tag=f"x{c}", name=f"x{c}")
                    nc.sync.dma_start(t_[:], xt_d[c * 128:(c + 1) * 128, 0:512])
                    x_r.setdefault(0, []).append(t_)
                for r in (0, 1):
                    if r == 1:
                        x_r[1] = []
                        for c in range(CC):
                            t_ = pxa.tile([128, 512], f16, tag=f"x{c}", name=f"x{c}")
                            nc.sync.dma_start(
                                t_[:], xt_d[c * 128:(c + 1) * 128, 512:1024])
                            x_r[1].append(t_)
                    # chunk-outer over 8 psum banks: chunk c usable on arrival
                    pq8 = [psA.tile([128, 512], f32, tag=f"b{f}", name=f"b{f}")
                           for f in range(8)]
                    for c in range(CC):
                        for f in range(8):
                            nc.tensor.matmul(
                                pq8[f][:], w_sb[c][:, f * 128:(f + 1) * 128],
                                x_r[r][c][:],
                                start=(c == 0), stop=(c == CC - 1),
                            )
                    for f in range(8):
                        nc.vector.tensor_scalar_add(
                            qk_sb[f][:, r * 512:(r + 1) * 512], pq8[f][:],
                            bqk_sb[:, f:f + 1],
                        )
                    for tl in range(4):
                        tg = r * 4 + tl
                        pv = psA.tile([128, 512], f32, tag=f"b{tl}", name=f"pv{tl}")
                        for c in range(CC):
                            nc.tensor.matmul(
                                pv[:], x_r[r][c][:, tl * 128:(tl + 1) * 128],
                                w_sb[c][:, 1024:1536],
                                start=(c == 0), stop=False,
                            )
                        nc.tensor.matmul(pv[:], ones_sb[:], bv_sb[:],
                                         start=False, stop=True)
                        nc.vector.tensor_copy(
                            v_sb[tg][:].rearrange("p (h e) -> p h e", e=65)[:, :, 0:64],
                            pv[:].rearrange("p (h e) -> p h e", e=64),
                        )

            # x for tokens 1024..2047 (ranges 2,3), used by the interleaved
            # QKV: two more generations of the xa pool's chunk tiles
            x23 = []
            for h, lo in enumerate((1024, 1536)):
                xs = []
                for c in range(CC):
                    t_ = pxa.tile([128, 512], f16, tag=f"x{c}", name=f"x{c}")
                    nc.sync.dma_start(t_[:], xt_d[c * 128:(c + 1) * 128, lo:lo + 512])
                    xs.append(t_)
                x23.append(xs)

            # ---------------- Phase 2: attention ----------------------------
            # Pair-units (half, fq, qc, kt): heads A=2fq (partitions 0-63)
            # and B=2fq+1 (64-127) computed together; their score MMs hit
            # disjoint PE row-groups and run concurrently.
            if True:
                def emit_scores(pss, u):
                    half, fq, qc, kt = u
                    qg = 1024 * half + 512 * qc
                    a = max(0, kt * 128 - qg)
                    diag = kt >= 8 * half + 4 * qc
                    kcol = slice(kt * 128, (kt + 1) * 128)
                    qcol = slice(qg + a, qg + 512)
                    ps = pss.tile([128, 1024], f32, tag="s", name="ps_s")
                    nc.tensor.matmul(
                        ps[0:128, a:512],
                        qk_sb[4 + fq][0:64, kcol], qk_sb[fq][0:64, qcol],
                        start=True, stop=not diag,
                    )
                    nc.tensor.matmul(
                        ps[0:128, 512 + a:1024],
                        qk_sb[4 + fq][64:128, kcol], qk_sb[fq][64:128, qcol],
                        start=True, stop=not diag,
                    )
                    if diag:
                        # += -1e30 * upper_strict on each head's diag block
                        nc.tensor.matmul(
                            ps[:, a:a + 128],
                            maskb_sb[:, 0:128], maskb_sb[:, 128:256],
                            start=False, stop=True,
                        )
                        nc.tensor.matmul(
                            ps[:, 512 + a:512 + a + 128],
                            maskb_sb[:, 0:128], maskb_sb[:, 128:256],
                            start=False, stop=True,
                        )
                    return ps

                def emit_exp(u, ps):
                    half, fq, qc, kt = u
                    qg = 1024 * half + 512 * qc
                    a = max(0, kt * 128 - qg)
                    # one instruction covers both heads' valid regions
                    # ([a,512) and [512+a,1024)); cols [512-a,512) of e are
                    # exp(stale PSUM) and are never streamed into y.
                    e = ep.tile([128, 1024], f16, tag="e", name="e_t")
                    nc.scalar.activation(
                        e[:, 0:1024 - a], ps[:, a:1024], EXP, scale=0.125,
                    )
                    return e

                def emit_y(psy, u, e):
                    half, fq, qc, kt = u
                    qg = 1024 * half + 512 * qc
                    a = max(0, kt * 128 - qg)
                    last = 8 * half + 4 * qc + 3
                    if kt == 0:
                        py_cur[(half, fq, qc)] = psy.tile(
                            [128, 1024], f32, tag="py", name="py_t")
                    py = py_cur[(half, fq, qc)]
                    nc.tensor.matmul(
                        py[0:65, a:512],
                        v_sb[kt][:, (2 * fq) * 65:(2 * fq + 1) * 65],
                        e[:, 0:512 - a],
                        start=(kt == 0), stop=(kt == last),
                    )
                    nc.tensor.matmul(
                        py[0:65, 512 + a:1024],
                        v_sb[kt][:, (2 * fq + 1) * 65:(2 * fq + 2) * 65],
                        e[:, 512:1024 - a],
                        start=(kt == 0), stop=(kt == last),
                    )

                def emit_norm(u):
                    half, fq, qc, _ = u
                    py = py_cur.pop((half, fq, qc))
                    # stage rowsum to SBUF: the custom-DVE recip's bit-trick
                    # seed misreads PSUM
                    rs = rp.tile([1, 1024], f32, tag="rs", name="rs_t")
                    nc.vector.tensor_copy(rs[:], py[64:65, 0:1024])
                    r = rp.tile([1, 1024], f32, tag="r", name="r_t")
                    nc.vector.reciprocal_approx_fast(r[:], rs[:])
                    rb = rp.tile([64, 1024], f32, tag="rb", name="rb_t")
                    nc.gpsimd.partition_broadcast(rb[:], r[:])
                    qcc = slice(qc * 512, qc * 512 + 512)
                    nc.vector.tensor_mul(
                        yn_cur[half][fq][0:64, qcc], py[0:64, 0:512], rb[:, 0:512],
                    )
                    nc.vector.tensor_mul(
                        yn_cur[half][fq][64:128, qcc], py[0:64, 512:1024],
                        rb[:, 512:1024],
                    )
                    if fq == 3 and qc == 1:
                        for tt in range(8):
                            proj_q.append((half, tt))

                def emit_proj_tt(pss, obp):
                    half, tt = proj_q.pop(0)
                    po = pss.tile([128, 1024], f32, tag="s", name="po_t")
                    for fc in range(4):
                        for n in range(2):
                            nc.tensor.matmul(
                                po[:, n * 512:(n + 1) * 512],
                                yn_cur[half][fc][:, tt * 128:(tt + 1) * 128],
                                wp_sb[fc][:, n * 512:(n + 1) * 512],
                                start=(fc == 0), stop=(fc == 3),
                            )
                    ob = obp.tile([128, C], f32, tag="ob")
                    nc.vector.tensor_copy(ob[:], po[:])
                    nc.sync.dma_start(
                        out_d[half * 1024 + tt * 128:half * 1024 + (tt + 1) * 128, :],
                        ob[:],
                    )

                def run_stream(pss, psy, units, fillers, obp=None, proj_every=12):
                    # scores(i+1) traced before y(i). Fillers (independent PE
                    # work) are emitted ONLY at pair-group boundaries: the psy
                    # pool slot they rotate through frees exactly there.
                    fq_ = list(fillers)
                    ps_i = emit_scores(pss, units[0])
                    for i, u in enumerate(units):
                        half, fq, qc, kt = u
                        if half not in yn_cur:
                            yn_cur[half] = [
                                ynp.tile([128, 1024], f16, tag=f"yn{fc}", name=f"yn{fc}")
                                for fc in range(4)
                            ]
                        e_i = emit_exp(u, ps_i)
                        if i + 1 < len(units):
                            ps_i = emit_scores(pss, units[i + 1])
                        emit_y(psy, u, e_i)
                        if kt == 8 * half + 4 * qc + 3:
                            emit_norm(u)
                            for _ in range(2):
                                if fq_:
                                    fq_.pop(0)(psy)
                        if obp is not None and proj_q and i % proj_every == 0:
                            emit_proj_tt(pss, obp)
                    while fq_:
                        fq_.pop(0)(psy)

                def make_units(half):
                    return [(half, fq, qc, kt)
                            for fq in range(4)
                            for qc in range(2)
                            for kt in range(8 * half + 4 * qc + 4)]

                units0 = make_units(0)
                units1 = make_units(1)

                # interleaved QKV work items for tokens 1024..2047
                fillers = []
                for f in range(8):
                    fillers.append(lambda p, f=f: emit_qk_feature(p, f, x23, 1024))
                for tl in range(8):
                    fillers.append(lambda p, tl=tl: emit_v_tile(p, tl, x23, 8 + tl))

                with (
                    tc.tile_pool(name="pss0", bufs=2, space="PSUM") as pss0,
                    tc.tile_pool(name="psy0", bufs=2, space="PSUM") as psy0,
                ):
                    run_stream(pss0, psy0, units0, fillers)

          # w and x23 pools closed; half 1 needs neither
          with (
              tc.tile_pool(name="obp", bufs=2) as obp,
              tc.tile_pool(name="pss1", bufs=2, space="PSUM") as pss1,
              tc.tile_pool(name="psy1", bufs=2, space="PSUM") as psy1,
          ):
              run_stream(pss1, psy1, units1, [], obp=obp, proj_every=12)
              while proj_q:
                  emit_proj_tt(pss1, obp)

    nc.compile()
    return nc


def _get_nc():
    if "nc" not in _CACHE:
        _CACHE["nc"] = _build_nc()
    return _CACHE["nc"]


def prepare_in_maps(x, W_attn, b_attn, W_proj, b_proj):
    import ml_dtypes
    x = np.asarray(x, dtype=np.float32)
    W_attn = np.asarray(W_attn, dtype=np.float32)
    b_attn = np.asarray(b_attn, dtype=np.float32)
    W_proj = np.asarray(W_proj, dtype=np.float32)

    mask = np.zeros((128, 256), np.float32)
    mask[:, 0:128] = np.triu(np.ones((128, 128), np.float32), 1)
    mask[:, 128:256] = -1e30 * np.eye(128, dtype=np.float32)
    maskb = np.ascontiguousarray(mask.astype(ml_dtypes.bfloat16))
    ones = np.ones((1, 128), np.float16)
    xts = [np.ascontiguousarray(x[b].T.astype(np.float16)) for b in range(4)]

    in_maps = []
    for c in range(8):
        b, hg = divmod(c, 2)
        s = hg * 512
        wqkv = np.ascontiguousarray(np.concatenate(
            [W_attn[:, s:s + 512],
             W_attn[:, 1024 + s:1024 + s + 512],
             W_attn[:, 2048 + s:2048 + s + 512]], axis=1).astype(np.float16))
        bqk = np.ascontiguousarray(
            np.concatenate([b_attn[s:s + 512], b_attn[1024 + s:1024 + s + 512]])
            .reshape(8, 128).T)
        bv = np.ascontiguousarray(
            b_attn[2048 + s:2048 + s + 512].reshape(1, 512).astype(np.float16))
        wproj = np.ascontiguousarray(
            W_proj[s:s + 512, :].astype(np.float16))
        in_maps.append({"xt": xts[b], "wqkv": wqkv, "bqk": bqk, "bv": bv,
                        "wproj": wproj, "ones": ones, "maskb": maskb})
    return in_maps


def kernel(x, W_attn, b_attn, W_proj, b_proj):
    from concourse.bass_utils import run_bass_kernel_spmd

    b_proj = np.asarray(b_proj, dtype=np.float32)
    nc = _get_nc()
    in_maps = prepare_in_maps(x, W_attn, b_attn, W_proj, b_proj)

    res = run_bass_kernel_spmd(nc, in_maps, core_ids=list(range(8)))
    y = np.empty((4, T, C), np.float32)
    for b in range(4):
        y[b] = res.results[2 * b]["out"] + res.results[2 * b + 1]["out"] + b_proj
    return y


# revision 9
# speedup vs baseline: 1.2576x; 1.0403x over previous
"""Causal self-attention (B=4, T=2048, C=1024, 16 heads) on 8 trn2 NeuronCores.

Sharding: core c handles batch b = c//2 and head-group hg = c%2 (8 of 16 heads).
Each core computes QKV projection for its heads, causal attention, and a partial
output projection (row-sharded W_proj); the host sums the two partials per batch
and adds b_proj.

Device layout notes:
 - x is fed pre-transposed ([C, T]) so the contraction dim C lands on SBUF
   partitions with no on-device transpose.
 - Scores are computed transposed (S^T[k, q]) so softmax's reduction over k can
   be done by the PE via a ones-column appended to V (row k of S^T is a
   partition; summing over partitions is a matmul).
 - Softmax skips the max-subtraction: scores/8 are ~N(0,1) here, exp is safe in
   fp32 and the result is mathematically identical.
 - All matmul operands are fp16 (fp32 PSUM accumulate): same PE stream rate as
   fp32, but FWL (fast weight load) halves LDWEIGHTS time, and SBUF/DMA
   traffic halves. fp16's 11-bit mantissa keeps end-to-end rel err ~3e-3.

Performance structure (v5), built around three engine limits measured in the
v4 trace: PE matmul streaming (~213ns per N=512), per-matmul LDWEIGHTS
serialization, and the ACT engine's exp cost ((N+352)/1.2 ns per instruction):
 - Scores matmuls have K=64 (head dim): the two heads of a feature-pair (fq)
   live on partitions 0-63 / 64-127, so their score MMs target disjoint PE
   row-groups (tile_position auto-derived from base_partition) and run
   CONCURRENTLY when issued back-to-back - halving scores PE time.
 - Attention is organized in pair-units (half, fq, qc, kt) where qc is a
   512-token q chunk: one PSUM tile [128, 1024] holds both heads' scores
   (A: cols 0-511, B: 512-1023), so ONE exp instruction covers two heads
   (fewer ACT fixed overheads). Unwritten diag-trim columns are exp'd as
   garbage but never streamed into the y matmuls.
 - y accumulates per pair-group into a [65, 1024] PSUM region (rows 0-63 y,
   row 64 rowsum via the V ones-column; A cols 0-511, B 512-1023).
 - Normalization per pair-group: rowsum staged to SBUF (the custom-DVE
   reciprocal's bit-trick seed misreads PSUM), reciprocal_approx_fast (DVE),
   partition_broadcast (GpSimd), then two DVE multiplies -> yn (fp16).
 - Phase 1a (QKV for tokens 0..1023): chunk-outer loop over 8 PSUM banks so
   the first matmul starts right after the first w/x chunk DMA lands.
 - QKV for tokens 1024..2047 is interleaved into half-0's attention stream
   (fillers at pair-group boundaries); output projection of half 0 drips into
   half 1's attention stream, half 1's projection is the tail. Output DMA
   overlaps compute. This keeps the PE queue full while ACT paces the
   attention stream, and keeps the PE HAM clock at 2.4GHz.
"""
import numpy as np

T = 2048          # tokens per batch element
C = 1024          # embed dim
H = 8             # heads per core
D = 64            # head dim
CC = 8            # contraction chunks (C / 128)

_CACHE = {}


def _build_nc():
    from concourse import bacc
    import concourse.mybir as mybir
    import concourse.tile as tile

    f32 = mybir.dt.float32
    f16 = mybir.dt.float16
    bf16 = mybir.dt.bfloat16
    EXP = mybir.ActivationFunctionType.Exp

    nc = bacc.Bacc("TRN2", num_devices=8, debug=False)

    xt_d = nc.dram_tensor("xt", [C, T], f16, kind="ExternalInput")
    wqkv_d = nc.dram_tensor("wqkv", [C, 1536], f16, kind="ExternalInput")
    bqk_d = nc.dram_tensor("bqk", [128, 8], f32, kind="ExternalInput")
    bv_d = nc.dram_tensor("bv", [1, 512], f16, kind="ExternalInput")
    wproj_d = nc.dram_tensor("wproj", [512, C], f16, kind="ExternalInput")
    ones_d = nc.dram_tensor("ones", [1, 128], f16, kind="ExternalInput")
    maskb_d = nc.dram_tensor("maskb", [128, 256], bf16, kind="ExternalInput")
    out_d = nc.dram_tensor("out", [T, C], f16, kind="ExternalOutput")

    with tile.TileContext(nc) as tc:
      with tc.tile_pool(name="persist", bufs=1) as pp:
        # persistent SBUF: qk^T [1024 feats, T] f16, v [T, 8*(64+1)] f16
        qk_sb = [pp.tile([128, T], f16, tag=f"qk{f}", name=f"qk{f}") for f in range(8)]
        v_sb = [pp.tile([128, H * 65], f16, tag=f"v{t}", name=f"v{t}") for t in range(16)]
        wp_sb = [pp.tile([128, C], f16, tag=f"wp{i}", name=f"wp{i}") for i in range(4)]
        maskb_sb = pp.tile([128, 256], bf16, tag="maskb")
        ones_sb = pp.tile([1, 128], f16, tag="ones")
        bqk_sb = pp.tile([128, 8], f32, tag="bqk")
        bv_sb = pp.tile([1, 512], f16, tag="bv")

        def persist_dmas():
            # issued AFTER the first w/x chunk DMAs: nothing here is needed
            # until attention / projection, so keep it off the critical path
            nc.sync.dma_start(maskb_sb[:], maskb_d[:])
            nc.sync.dma_start(bqk_sb[:], bqk_d[:])
            for i in range(4):
                nc.sync.dma_start(wp_sb[i][:], wproj_d[i * 128:(i + 1) * 128, :])

        for t in range(16):
            # ones column at position 64 of each head's 65-wide V block
            nc.gpsimd.memset(
                v_sb[t][:].rearrange("p (h e) -> p h e", e=65)[:, :, 64:65], 1.0
            )

        def emit_qk_feature(pool, f, xs2, dst):
            # q/k features f*128..f*128+128 for tokens dst..dst+1024
            # (xs2 = two lists of 512-col x chunk tiles)
            pq = pool.tile([128, 1024], f32, tag="py", name="pq")
            for h, xs in enumerate(xs2):
                for c in range(CC):
                    nc.tensor.matmul(
                        pq[:, h * 512:(h + 1) * 512],
                        w_sb[c][:, f * 128:(f + 1) * 128],
                        xs[c][:],
                        start=(c == 0), stop=(c == CC - 1),
                    )
            nc.vector.tensor_scalar_add(
                qk_sb[f][:, dst:dst + 1024], pq[:], bqk_sb[:, f:f + 1]
            )

        def emit_v_tile(pool, tl, xs2, tg):
            # v for 128 tokens (tl-th 128-block of xs2) -> v_sb[tg]
            xs = xs2[tl // 4]
            t0 = (tl % 4) * 128
            pv = pool.tile([128, 1024], f32, tag="py", name="pv")
            for c in range(CC):
                nc.tensor.matmul(
                    pv[:, 0:512], xs[c][:, t0:t0 + 128],
                    w_sb[c][:, 1024:1536],
                    start=(c == 0), stop=False,
                )
            nc.tensor.matmul(pv[:, 0:512], ones_sb[:], bv_sb[:], start=False, stop=True)
            nc.vector.tensor_copy(
                v_sb[tg][:].rearrange("p (h e) -> p h e", e=65)[:, :, 0:64],
                pv[:, 0:512].rearrange("p (h e) -> p h e", e=64),
            )

        # ---------------- Phase 1a: QKV for tokens 0..1023 (ranges 0,1) -----
        with (
            tc.tile_pool(name="ynp", bufs=2) as ynp,
            tc.tile_pool(name="epool", bufs=3) as ep,
            tc.tile_pool(name="rpool", bufs=2) as rp,
        ):
          yn_cur = {}
          py_cur = {}
          proj_q = []
          with (
            tc.tile_pool(name="w", bufs=1) as pw,
            tc.tile_pool(name="xa", bufs=2) as pxa,
          ):
            w_sb = [pw.tile([128, 1536], f16, tag=f"w{c}", name=f"w{c}") for c in range(CC)]
            with (
                tc.tile_pool(name="psA", bufs=1, space="PSUM") as psA,
            ):
                x_r = {}
                for c in range(CC):
                    # pair chunk DMAs so the first matmul group starts early
                    nc.sync.dma_start(w_sb[c][:], wqkv_d[c * 128:(c + 1) * 128, :])
                    t_ = pxa.tile([128, 512], f16, tag=f"x{c}", name=f"x{c}")
                    nc.sync.dma_start(t_[:], xt_d[c * 128:(c + 1) * 128, 0:512])
                    x_r.setdefault(0, []).append(t_)
                    if c == 0:
                        # tiny, needed a few us in by the v-tile bias matmul
                        nc.sync.dma_start(ones_sb[:], ones_d[:])
                        nc.sync.dma_start(bv_sb[:], bv_d[:])
                    if c == 3:
                        persist_dmas()
                for r in (0, 1):
                    if r == 1:
                        x_r[1] = []
                        for c in range(CC):
                            t_ = pxa.tile([128, 512], f16, tag=f"x{c}", name=f"x{c}")
                            nc.sync.dma_start(
                                t_[:], xt_d[c * 128:(c + 1) * 128, 512:1024])
                            x_r[1].append(t_)
                    # chunk-outer over 8 psum banks: chunk c usable on arrival
                    pq8 = [psA.tile([128, 512], f32, tag=f"b{f}", name=f"b{f}")
                           for f in range(8)]
                    for c in range(CC):
                        for f in range(8):
                            nc.tensor.matmul(
                                pq8[f][:], w_sb[c][:, f * 128:(f + 1) * 128],
                                x_r[r][c][:],
                                start=(c == 0), stop=(c == CC - 1),
                            )
                    for f in range(8):
                        nc.vector.tensor_scalar_add(
                            qk_sb[f][:, r * 512:(r + 1) * 512], pq8[f][:],
                            bqk_sb[:, f:f + 1],
                        )
                    for tl in range(4):
                        tg = r * 4 + tl
                        pv = psA.tile([128, 512], f32, tag=f"b{tl}", name=f"pv{tl}")
                        for c in range(CC):
                            nc.tensor.matmul(
                                pv[:], x_r[r][c][:, tl * 128:(tl + 1) * 128],
                                w_sb[c][:, 1024:1536],
                                start=(c == 0), stop=False,
                            )
                        nc.tensor.matmul(pv[:], ones_sb[:], bv_sb[:],
                                         start=False, stop=True)
                        nc.vector.tensor_copy(
                            v_sb[tg][:].rearrange("p (h e) -> p h e", e=65)[:, :, 0:64],
                            pv[:].rearrange("p (h e) -> p h e", e=64),
                        )

            # x for tokens 1024..2047 (ranges 2,3), used by the interleaved
            # QKV: two more generations of the xa pool's chunk tiles
            x23 = []
            for h, lo in enumerate((1024, 1536)):
                xs = []
                for c in range(CC):
                    t_ = pxa.tile([128, 512], f16, tag=f"x{c}", name=f"x{c}")
                    nc.sync.dma_start(t_[:], xt_d[c * 128:(c + 1) * 128, lo:lo + 512])
                    xs.append(t_)
                x23.append(xs)

            # ---------------- Phase 2: attention ----------------------------
            # Pair-units (half, fq, qc, kt): heads A=2fq (partitions 0-63)
            # and B=2fq+1 (64-127) computed together; their score MMs hit
            # disjoint PE row-groups and run concurrently.
            if True:
                def emit_scores(pss, u):
                    half, fq, qc, kt = u
                    qg = 1024 * half + 512 * qc
                    a = max(0, kt * 128 - qg)
                    diag = kt >= 8 * half + 4 * qc
                    kcol = slice(kt * 128, (kt + 1) * 128)
                    qcol = slice(qg + a, qg + 512)
                    ps = pss.tile([128, 1024], f32, tag="s", name="ps_s")
                    nc.tensor.matmul(
                        ps[0:128, a:512],
                        qk_sb[4 + fq][0:64, kcol], qk_sb[fq][0:64, qcol],
                        start=True, stop=not diag,
                    )
                    nc.tensor.matmul(
                        ps[0:128, 512 + a:1024],
                        qk_sb[4 + fq][64:128, kcol], qk_sb[fq][64:128, qcol],
                        start=True, stop=not diag,
                    )
                    if diag:
                        # += -1e30 * upper_strict on each head's diag block
                        nc.tensor.matmul(
                            ps[:, a:a + 128],
                            maskb_sb[:, 0:128], maskb_sb[:, 128:256],
                            start=False, stop=True,
                        )
                        nc.tensor.matmul(
                            ps[:, 512 + a:512 + a + 128],
                            maskb_sb[:, 0:128], maskb_sb[:, 128:256],
                            start=False, stop=True,
                        )
                    return ps

                def emit_exp(u, ps):
                    half, fq, qc, kt = u
                    qg = 1024 * half + 512 * qc
                    a = max(0, kt * 128 - qg)
                    # one instruction covers both heads' valid regions
                    # ([a,512) and [512+a,1024)); cols [512-a,512) of e are
                    # exp(stale PSUM) and are never streamed into y.
                    e = ep.tile([128, 1024], f16, tag="e", name="e_t")
                    nc.scalar.activation(
                        e[:, 0:1024 - a], ps[:, a:1024], EXP, scale=0.125,
                    )
                    return e

                def emit_y(psy, u, e):
                    half, fq, qc, kt = u
                    qg = 1024 * half + 512 * qc
                    a = max(0, kt * 128 - qg)
                    last = 8 * half + 4 * qc + 3
                    if kt == 0:
                        py_cur[(half, fq, qc)] = psy.tile(
                            [128, 1024], f32, tag="py", name="py_t")
                    py = py_cur[(half, fq, qc)]
                    nc.tensor.matmul(
                        py[0:65, a:512],
                        v_sb[kt][:, (2 * fq) * 65:(2 * fq + 1) * 65],
                        e[:, 0:512 - a],
                        start=(kt == 0), stop=(kt == last),
                    )
                    nc.tensor.matmul(
                        py[0:65, 512 + a:1024],
                        v_sb[kt][:, (2 * fq + 1) * 65:(2 * fq + 2) * 65],
                        e[:, 512:1024 - a],
                        start=(kt == 0), stop=(kt == last),
                    )

                def emit_norm(u):
                    half, fq, qc, _ = u
                    py = py_cur.pop((half, fq, qc))
                    # stage rowsum to SBUF: the custom-DVE recip's bit-trick
                    # seed misreads PSUM
                    rs = rp.tile([1, 1024], f32, tag="rs", name="rs_t")
                    nc.vector.tensor_copy(rs[:], py[64:65, 0:1024])
                    r = rp.tile([1, 1024], f32, tag="r", name="r_t")
                    nc.vector.reciprocal_approx_fast(r[:], rs[:])
                    rb = rp.tile([64, 1024], f32, tag="rb", name="rb_t")
                    nc.gpsimd.partition_broadcast(rb[:], r[:])
                    qcc = slice(qc * 512, qc * 512 + 512)
                    nc.vector.tensor_mul(
                        yn_cur[half][fq][0:64, qcc], py[0:64, 0:512], rb[:, 0:512],
                    )
                    nc.vector.tensor_mul(
                        yn_cur[half][fq][64:128, qcc], py[0:64, 512:1024],
                        rb[:, 512:1024],
                    )
                    if fq == 3 and qc == 1:
                        for tt in range(8):
                            proj_q.append((half, tt))

                def emit_proj_tt(pss, obp):
                    half, tt = proj_q.pop(0)
                    po = pss.tile([128, 1024], f32, tag="s", name="po_t")
                    for fc in range(4):
                        for n in range(2):
                            nc.tensor.matmul(
                                po[:, n * 512:(n + 1) * 512],
                                yn_cur[half][fc][:, tt * 128:(tt + 1) * 128],
                                wp_sb[fc][:, n * 512:(n + 1) * 512],
                                start=(fc == 0), stop=(fc == 3),
                            )
                    ob = obp.tile([128, C], f16, tag="ob")
                    nc.vector.tensor_copy(ob[:], po[:])
                    nc.sync.dma_start(
                        out_d[half * 1024 + tt * 128:half * 1024 + (tt + 1) * 128, :],
                        ob[:],
                    )

                def run_stream(pss, psy, units, fillers, obp=None,
                               proj_every=12, per_end=2):
                    # scores(i+1) AND exp(i+1) are traced before y(i): the
                    # queued exp keeps ACT fed through PE bursts (fillers,
                    # proj), and decouples the pss slot rotation from the
                    # proj unit's DVE drain. Fillers (independent PE work)
                    # are emitted ONLY at pair-group boundaries: the psy pool
                    # slot they rotate through frees exactly there.
                    fq_ = list(fillers)
                    ps_i = emit_scores(pss, units[0])
                    e_i = emit_exp(units[0], ps_i)
                    for i, u in enumerate(units):
                        half, fq, qc, kt = u
                        if half not in yn_cur:
                            yn_cur[half] = [
                                ynp.tile([128, 1024], f16, tag=f"yn{fc}", name=f"yn{fc}")
                                for fc in range(4)
                            ]
                        if i + 1 < len(units):
                            ps_n = emit_scores(pss, units[i + 1])
                            e_n = emit_exp(units[i + 1], ps_n)
                        emit_y(psy, u, e_i)
                        if kt == 8 * half + 4 * qc + 3:
                            emit_norm(u)
                            for _ in range(per_end):
                                if fq_:
                                    fq_.pop(0)(psy)
                        if obp is not None and proj_q and i % proj_every == 0:
                            emit_proj_tt(pss, obp)
                        if i + 1 < len(units):
                            e_i = e_n
                    while fq_:
                        fq_.pop(0)(psy)

                def make_units(half):
                    if half == 0:
                        return [(0, fq, qc, kt)
                                for fq in range(4)
                                for qc in range(2)
                                for kt in range(4 * qc + 4)]
                    # half 1: qc-outer so the v12-15 fillers (emitted at the
                    # first group boundaries) land before any kt>=12 unit
                    return [(1, fq, qc, kt)
                            for qc in range(2)
                            for fq in range(4)
                            for kt in range(8 + 4 * qc + 4)]

                units0 = make_units(0)
                units1 = make_units(1)

                # interleaved QKV work items for tokens 1024..2047.
                # q/k features ordered (0,4),(1,5),.. so half-1's fq-ordered
                # groups see their features early; v12-15 move to half-1's
                # stream (only kt>=12 units need them).
                fillers0 = []
                for f in (0, 4, 1, 5, 2, 6, 3, 7):
                    fillers0.append(lambda p, f=f: emit_qk_feature(p, f, x23, 1024))
                for tl in range(4):
                    fillers0.append(lambda p, tl=tl: emit_v_tile(p, tl, x23, 8 + tl))
                fillers1 = []
                for tl in range(4, 8):
                    fillers1.append(lambda p, tl=tl: emit_v_tile(p, tl, x23, 8 + tl))

                with (
                    tc.tile_pool(name="pss0", bufs=2, space="PSUM") as pss0,
                    tc.tile_pool(name="psy0", bufs=2, space="PSUM") as psy0,
                ):
                    run_stream(pss0, psy0, units0, fillers0)

                with (
                    tc.tile_pool(name="obp", bufs=2) as obp,
                    tc.tile_pool(name="pss1", bufs=2, space="PSUM") as pss1,
                    tc.tile_pool(name="psy1", bufs=2, space="PSUM") as psy1,
                ):
                    run_stream(pss1, psy1, units1, fillers1, obp=obp,
                               proj_every=12, per_end=1)
                    while proj_q:
                        emit_proj_tt(pss1, obp)

    nc.compile()
    return nc


def _get_nc():
    if "nc" not in _CACHE:
        _CACHE["nc"] = _build_nc()
    return _CACHE["nc"]


def prepare_in_maps(x, W_attn, b_attn, W_proj, b_proj):
    import ml_dtypes
    x = np.asarray(x, dtype=np.float32)
    W_attn = np.asarray(W_attn, dtype=np.float32)
    b_attn = np.asarray(b_attn, dtype=np.float32)
    W_proj = np.asarray(W_proj, dtype=np.float32)

    mask = np.zeros((128, 256), np.float32)
    mask[:, 0:128] = np.triu(np.ones((128, 128), np.float32), 1)
    mask[:, 128:256] = -1e30 * np.eye(128, dtype=np.float32)
    maskb = np.ascontiguousarray(mask.astype(ml_dtypes.bfloat16))
    ones = np.ones((1, 128), np.float16)
    xts = [np.ascontiguousarray(x[b].T.astype(np.float16)) for b in range(4)]

    in_maps = []
    for c in range(8):
        b, hg = divmod(c, 2)
        s = hg * 512
        wqkv = np.ascontiguousarray(np.concatenate(
            [W_attn[:, s:s + 512],
             W_attn[:, 1024 + s:1024 + s + 512],
             W_attn[:, 2048 + s:2048 + s + 512]], axis=1).astype(np.float16))
        bqk = np.ascontiguousarray(
            np.concatenate([b_attn[s:s + 512], b_attn[1024 + s:1024 + s + 512]])
            .reshape(8, 128).T)
        bv = np.ascontiguousarray(
            b_attn[2048 + s:2048 + s + 512].reshape(1, 512).astype(np.float16))
        wproj = np.ascontiguousarray(
            W_proj[s:s + 512, :].astype(np.float16))
        in_maps.append({"xt": xts[b], "wqkv": wqkv, "bqk": bqk, "bv": bv,
                        "wproj": wproj, "ones": ones, "maskb": maskb})
    return in_maps


def kernel(x, W_attn, b_attn, W_proj, b_proj):
    from concourse.bass_utils import run_bass_kernel_spmd

    b_proj = np.asarray(b_proj, dtype=np.float32)
    nc = _get_nc()
    in_maps = prepare_in_maps(x, W_attn, b_attn, W_proj, b_proj)

    res = run_bass_kernel_spmd(nc, in_maps, core_ids=list(range(8)))
    y = np.empty((4, T, C), np.float32)
    for b in range(4):
        y[b] = (res.results[2 * b]["out"].astype(np.float32)
                + res.results[2 * b + 1]["out"].astype(np.float32) + b_proj)
    return y
